# revision 10
# baseline (speedup 1.0000x reference)
"""AugmentedConv attention kernel for 8 TRN2 NeuronCores.

Sharding: core c = 2*b + g  (b = batch 0..3, g = head-group 0..1, 4 heads each).
Each core: 3x conv3x3 (its 384 qkv output channels, full 256-ch contraction),
per-head attention with relative-position logits folded into the logit matmul
as extra accumulating passes (one-hot lhsT), softmax without max-subtraction
(logits are O(+-8)), AV + row-sum matmuls, 1x1 out-conv partial over its 128
v-channels.  Host sums the two partials per batch and adds b_o1.

Layouts (per core):
  spatial index s = y*32 + x  (row-major, matches reference H*W flatten)
  q,k stored [c=128 (4 heads x 32), s=1024] channel-major (bf16)
  v transposed on-device to [s, c] for the AV contraction
  logits computed TRANSPOSED as l[k, q] so that softmax-sum and AV contract
  over the partition dim via matmuls (row-tiled, 4 heads packed).
"""

import sys

sys.path.insert(0, "/opt/trn_rl_repo")

import numpy as np
import ml_dtypes

BF16 = ml_dtypes.bfloat16

B, C, H, W = 4, 256, 32, 32
DK = DV = 256
NH = 8
DKH = DVH = 32
K = 3
HW = H * W  # 1024
NCORES = 8
HEADS_PER_CORE = 4
SCALE = DKH ** -0.5

_BUILt = {}


def _build_nc():
    import concourse.bass as bass
    import concourse.mybir as mybir
    import concourse.tile as tile
    from concourse import bacc
    from concourse.masks import make_identity

    f32 = mybir.dt.float32
    bf16 = mybir.dt.bfloat16
    AF = mybir.ActivationFunctionType
    ALU = mybir.AluOpType

    nc = bacc.Bacc("TRN2", target_bir_lowering=False, debug=False,
                   num_devices=NCORES)

    # ---- external I/O (per-core shards, host-prepped) ----
    xp_d = nc.declare_dram_parameter("xp", [3, 2, 128, 34 * 34], bf16, isOutput=False)
    wt_d = nc.declare_dram_parameter("wt", [3, 2, 128, 9, 384], bf16, isOutput=False)
    bqkv_d = nc.declare_dram_parameter("bqkv", [128, 3, 3], f32, isOutput=False)
    woT_d = nc.declare_dram_parameter("woT", [128, 256], bf16, isOutput=False)
    relh_d = nc.declare_dram_parameter("relh", [32, 128, 32], bf16, isOutput=False)
    relw_d = nc.declare_dram_parameter("relw", [32, 128, 32], bf16, isOutput=False)
    ohy_d = nc.declare_dram_parameter("ohy", [128, 1024], bf16, isOutput=False)
    ohx_d = nc.declare_dram_parameter("ohx", [128, 1024], bf16, isOutput=False)
    o_d = nc.declare_dram_parameter("o", [3, 256, 1024], f32, isOutput=True)

    with tile.TileContext(nc) as tc:
        with (
            tc.tile_pool(name="const", bufs=1) as const,
            tc.tile_pool(name="work", bufs=1) as work,
            tc.tile_pool(name="cpsum", bufs=2, space="PSUM") as cpsum,
            tc.tile_pool(name="lpsum", bufs=1, space="PSUM") as lpsum,
            tc.tile_pool(name="apsum", bufs=1, space="PSUM") as apsum,
            tc.tile_pool(name="expp", bufs=4) as expp,
            tc.tile_pool(name="outp", bufs=2) as outp,
            tc.tile_pool(name="dscr", bufs=1, space="DRAM") as dscr,
        ):
            # ---- constants ----
            bqkv = const.tile([128, 3, 3], f32)
            nc.sync.dma_start(bqkv[:], bqkv_d[:])
            woT = const.tile([128, 256], bf16)
            nc.sync.dma_start(woT[:], woT_d[:])
            relh = const.tile([128, 32, 32], bf16)
            nc.sync.dma_start(relh[:], relh_d[:].rearrange("y p m -> p y m"))
            relw = const.tile([128, 32, 32], bf16)
            nc.sync.dma_start(relw[:], relw_d[:].rearrange("y p m -> p y m"))
            ohy = const.tile([128, 1024], bf16)
            nc.sync.dma_start(ohy[:], ohy_d[:])
            ohx = const.tile([128, 1024], bf16)
            nc.sync.dma_start(ohx[:], ohx_d[:])
            ident = const.tile([128, 128], bf16)
            make_identity(nc, ident[:])
            ones32 = const.tile([128, 32], bf16)
            nc.vector.memset(ones32[:], 1.0)

            xp = const.tile([128, 3, 2, 34 * 34], bf16)
            wt = const.tile([128, 3, 2, 9, 384], bf16)
            xp_loaded = [False] * 3

            def load_t(t):
                if not xp_loaded[t]:
                    xp_loaded[t] = True
                    for cib in range(2):
                        nc.sync.dma_start(xp[:, t, cib], xp_d[t, cib])
                        nc.sync.dma_start(wt[:, t, cib], wt_d[t, cib])

            qkvt = [[work.tile([128, 1024], bf16, name=f"qkv_{t}_{ct}")
                     for ct in range(3)] for t in range(3)]
            vTt = [work.tile([128, 8, 4, 32], bf16, name=f"vT_{t}")
                   for t in range(3)]
            Rh = work.tile([128, 1024], bf16)
            Rw = work.tile([128, 1024], bf16)
            kd = work.tile([128, 1024], bf16)
            scr = dscr.tile([3, 4, 32, 32, 32], bf16)

            xpv = xp[:].rearrange("p t c (r q) -> p t c r q", q=34)

            def conv_job(t, cot, qt):
                load_t(t)
                ps = cpsum.tile([128, 512], f32, tag="conv")
                n = 0
                for cib in range(2):
                    for ky in range(3):
                        for kx in range(3):
                            rhs = xpv[:, t, cib,
                                      qt * 16 + ky: qt * 16 + ky + 16,
                                      kx: kx + 32]
                            nc.tensor.matmul(
                                ps[:],
                                lhsT=wt[:, t, cib, ky * 3 + kx,
                                        cot * 128:(cot + 1) * 128],
                                rhs=rhs,
                                start=(n == 0), stop=(n == 17),
                            )
                            n += 1
                nc.vector.tensor_tensor(
                    out=qkvt[t][cot][:, qt * 512:(qt + 1) * 512],
                    in0=ps[:],
                    in1=bqkv[:, t, cot, None].to_broadcast((128, 512)),
                    op=ALU.add,
                )

            def vt_job(t, half):
                for kt in range(4 * half, 4 * half + 4):
                    pst = cpsum.tile([128, 128], bf16, tag="conv")
                    nc.tensor.transpose(
                        pst[:],
                        qkvt[t][2][:, kt * 128:(kt + 1) * 128],
                        ident[:],
                    )
                    nc.vector.tensor_copy(
                        out=vTt[t][:, kt, :, :].rearrange("p a b -> p (a b)"),
                        in_=pst[:],
                    )

            def rel_job(kind, j):
                """one head's rel table: 2x16-col-group accumulation sweeps"""
                q1 = qkvt[0][0][:]
                q1v = qkvt[0][0][:].rearrange("p (y x) -> p y x", x=32)
                Rwv = Rw[:].rearrange("p (y x) -> p x y", x=32)
                for half in range(2):
                    ps = cpsum.tile([128, 512], f32, tag="conv")
                    for yl in range(16):
                        y = half * 16 + yl
                        if kind == 0:
                            lhs = relh[32 * j:32 * j + 32, y, :]
                            rhs = q1[32 * j:32 * j + 32, y * 32:y * 32 + 32]
                        else:
                            lhs = relw[32 * j:32 * j + 32, y, :]
                            rhs = q1v[32 * j:32 * j + 32, :, y]
                        nc.tensor.matmul(
                            ps[32 * j:32 * j + 32, yl * 32:yl * 32 + 32],
                            lhsT=lhs, rhs=rhs,
                            tile_position=(32 * j, 32 * j),
                            start=(yl == 0), stop=(yl == 15),
                        )
                    if kind == 0:
                        nc.vector.tensor_copy(
                            out=Rh[32 * j:32 * j + 32,
                                   half * 512:half * 512 + 512],
                            in_=ps[32 * j:32 * j + 32, :],
                        )
                    else:
                        nc.vector.tensor_copy(
                            out=Rwv[32 * j:32 * j + 32,
                                    half * 16:half * 16 + 16, :],
                            in_=ps[32 * j:32 * j + 32, :].rearrange(
                                "p (a b) -> p a b", b=32),
                        )

            def kd_job():
                nc.vector.tensor_tensor(out=kd[:], in0=qkvt[0][1][:],
                                        in1=qkvt[1][1][:], op=ALU.subtract)

            def attention(br, qs, ks, vt, userel, fillers):
                fi = 0
                for qt in range(2):
                    pa = apsum.tile([128, 512], f32, tag="av")
                    psum_s = apsum.tile([128, 512], f32, tag="sums")
                    for kt in range(8):
                        pl = lpsum.tile([128, 4, 512], f32, tag="logits")
                        for j in range(4):
                            nc.tensor.matmul(
                                pl[:, j, :],
                                lhsT=ks[32 * j:32 * j + 32,
                                        kt * 128:(kt + 1) * 128],
                                rhs=qs[32 * j:32 * j + 32,
                                       qt * 512:(qt + 1) * 512],
                                tile_position=(32 * j, 0),
                                start=True, stop=not userel,
                            )
                        if userel:
                            for j in range(4):
                                nc.tensor.matmul(
                                    pl[:, j, :],
                                    lhsT=ohy[32 * j:32 * j + 32,
                                             kt * 128:(kt + 1) * 128],
                                    rhs=Rh[32 * j:32 * j + 32,
                                           qt * 512:(qt + 1) * 512],
                                    tile_position=(32 * j, 0),
                                    start=False, stop=False,
                                )
                            for j in range(4):
                                nc.tensor.matmul(
                                    pl[:, j, :],
                                    lhsT=ohx[32 * j:32 * j + 32,
                                             kt * 128:(kt + 1) * 128],
                                    rhs=Rw[32 * j:32 * j + 32,
                                           qt * 512:(qt + 1) * 512],
                                    tile_position=(32 * j, 0),
                                    start=False, stop=True,
                                )
                        for hp in range(2):
                            ex = expp.tile([128, 2, 512], bf16, tag="exp")
                            nc.scalar.activation(
                                ex[:], pl[:, 2 * hp:2 * hp + 2, :], AF.Exp)
                            for jl, j in enumerate((2 * hp, 2 * hp + 1)):
                                nc.tensor.matmul(
                                    pa[32 * j:32 * j + 32, :],
                                    lhsT=vTt[vt][:, kt, j, :],
                                    rhs=ex[:, jl, :],
                                    tile_position=(0, 32 * j),
                                    start=(kt == 0), stop=(kt == 7),
                                )
                                nc.tensor.matmul(
                                    psum_s[32 * j:32 * j + 32, :],
                                    lhsT=ones32[:],
                                    rhs=ex[:, jl, :],
                                    tile_position=(0, 32 * j),
                                    start=(kt == 0), stop=(kt == 7),
                                )
                        # filler work for PE while ScalarE runs exp
                        if fi < len(fillers):
                            fillers[fi]()
                            fi += 1
                    recip = outp.tile([128, 512], f32, tag="recip")
                    nc.vector.reciprocal_approx_fast(out=recip[:],
                                                     in_=psum_s[:])
                    a_sb = outp.tile([128, 512], bf16, tag="asb")
                    nc.vector.tensor_tensor(out=a_sb[:], in0=pa[:],
                                            in1=recip[:], op=ALU.mult)
                    nc.sync.dma_start(
                        out=scr[br].rearrange("n x dp y -> (n x) dp y")[
                            :, qt * 16:qt * 16 + 16, :],
                        in_=a_sb[:].rearrange("p (dp y) -> p dp y", y=32),
                    )
                while fi < len(fillers):
                    fillers[fi]()
                    fi += 1
                G = outp.tile([128, 1024], bf16, tag="G")
                for n in range(4):
                    nc.sync.dma_start(
                        out=G[32 * n:32 * n + 32, :].rearrange(
                            "p (x y) -> p x y", y=32),
                        in_=scr[br, n].rearrange("x dp y -> dp x y"),
                    )
                for qt2 in range(2):
                    for cot in range(2):
                        po = cpsum.tile([128, 512], f32, tag="conv")
                        nc.tensor.matmul(
                            po[:], lhsT=woT[:, cot * 128:(cot + 1) * 128],
                            rhs=G[:, qt2 * 512:(qt2 + 1) * 512],
                            start=True, stop=True)
                        osb = outp.tile([128, 512], f32, tag="osb")
                        nc.vector.tensor_copy(out=osb[:], in_=po[:])
                        nc.sync.dma_start(
                            out=o_d[br, cot * 128:(cot + 1) * 128,
                                    qt2 * 512:(qt2 + 1) * 512],
                            in_=osb[:],
                        )

            # ---- schedule: prologue convs, then branches 1, 2, 0 with
            # remaining conv/rel work as exp-gap fillers ----
            for t, cot in [(1, 0), (2, 1), (1, 2)]:
                for qt in range(2):
                    conv_job(t, cot, qt)
            vt_job(1, 0)
            vt_job(1, 1)

            br1_fill = [
                lambda: conv_job(0, 0, 0), lambda: conv_job(0, 0, 1),  # q1
                lambda: conv_job(0, 1, 0), lambda: conv_job(0, 1, 1),  # k1
                lambda: conv_job(1, 1, 0), lambda: conv_job(1, 1, 1),  # k2
                kd_job,
                lambda: conv_job(2, 0, 0), lambda: conv_job(2, 0, 1),  # q12
                lambda: conv_job(2, 2, 0), lambda: conv_job(2, 2, 1),  # v12
                lambda: vt_job(2, 0), lambda: vt_job(2, 1),
                lambda: rel_job(0, 0), lambda: rel_job(0, 1),
                lambda: rel_job(0, 2),
            ]
            attention(1, qkvt[1][0][:], qkvt[2][1][:], 1, False, br1_fill)

            br2_fill = [
                lambda: rel_job(0, 3),
                lambda: rel_job(1, 0), lambda: rel_job(1, 1),
                lambda: rel_job(1, 2), lambda: rel_job(1, 3),
                lambda: conv_job(0, 2, 0), lambda: conv_job(0, 2, 1),  # v1
                lambda: vt_job(0, 0), lambda: vt_job(0, 1),
            ]
            attention(2, qkvt[2][0][:], kd[:], 2, False, br2_fill)

            attention(0, qkvt[0][0][:], qkvt[2][1][:], 0, True, [])

    nc.compile()
    return nc


def _host_prep(inputs):
    """Build per-core in_maps."""
    x_all = [inputs["x1"], inputs["x2"], inputs["x12"]]
    w_all = [inputs["W_qkv1"], inputs["W_qkv2"], inputs["W_qkv12"]]
    b_all = [inputs["b_qkv1"], inputs["b_qkv2"], inputs["b_qkv12"]]
    W_o1 = np.asarray(inputs["W_o1"], np.float32)
    key_rel_w = np.asarray(inputs["key_rel_w"], np.float32)
    key_rel_h = np.asarray(inputs["key_rel_h"], np.float32)

    # padded inputs, bf16: [B, 2, 128, 34*34]
    xp_full = []
    for xt in x_all:
        xt = np.asarray(xt, np.float32)
        xpad = np.zeros((B, C, 34, 34), np.float32)
        xpad[:, :, 1:33, 1:33] = xt
        xp_full.append(xpad.reshape(B, 2, 128, 34 * 34))
    xp_full = np.stack(xp_full, axis=1)  # [B, 3, 2, 128, 1156]

    # one-hots / e4, shared by all cores
    kk = np.arange(1024)
    ohy = np.zeros((128, 1024), np.float32)
    ohx = np.zeros((128, 1024), np.float32)
    for j in range(4):
        for i in range(32):
            ohy[32 * j + i] = (kk // 32 == i)
            ohx[32 * j + i] = (kk % 32 == i)
    # rel tables (shared): relh[y, 32j+d, i] = key_rel_h[i - y + 31, d]
    relh = np.zeros((32, 128, 32), np.float32)
    relw = np.zeros((32, 128, 32), np.float32)
    for y in range(32):
        blk_h = key_rel_h[31 - y:63 - y, :].T  # [d=32, i=32]
        blk_w = key_rel_w[31 - y:63 - y, :].T
        for j in range(4):
            relh[y, 32 * j:32 * j + 32, :] = blk_h
            relw[y, 32 * j:32 * j + 32, :] = blk_w

    in_maps = []
    for c in range(NCORES):
        b, g = divmod(c, 2)
        # conv weights: 384 rows = (q128 scaled, k128, v128) for head group g
        wts = []
        bias = np.zeros((128, 3, 3), np.float32)
        for t in range(3):
            wf = np.asarray(w_all[t], np.float32)
            bf = np.asarray(b_all[t], np.float32)
            wq = wf[128 * g:128 * g + 128] * SCALE
            wk = wf[256 + 128 * g:256 + 128 * g + 128]
            wv = wf[512 + 128 * g:512 + 128 * g + 128]
            wsl = np.concatenate([wq, wk, wv], 0)  # [384, 256, 3, 3]
            # -> [2 cib, 128 ci, 9 tap, 384 co]
            wts.append(wsl.reshape(384, 2, 128, 9).transpose(1, 2, 3, 0))
            bias[:, t, 0] = bf[128 * g:128 * g + 128] * SCALE
            bias[:, t, 1] = bf[256 + 128 * g:256 + 128 * g + 128]
            bias[:, t, 2] = bf[512 + 128 * g:512 + 128 * g + 128]
        wt = np.stack(wts, 0)  # [3, 2, 128, 9, 384]
        woT = W_o1[:, 128 * g:128 * g + 128, 0, 0].T  # [128 dv, 256 co]
        in_maps.append({
            "xp": xp_full[b].astype(BF16),
            "wt": wt.astype(BF16),
            "bqkv": bias,
            "woT": np.ascontiguousarray(woT).astype(BF16),
            "relh": relh.astype(BF16),
            "relw": relw.astype(BF16),
            "ohy": ohy.astype(BF16),
            "ohx": ohx.astype(BF16),
        })
    return in_maps


def kernel(**inputs):
    from concourse.bass_utils import run_bass_kernel_spmd

    if "nc" not in _BUILt:
        _BUILt["nc"] = _build_nc()
    nc = _BUILt["nc"]

    in_maps = _host_prep(inputs)
    res = run_bass_kernel_spmd(nc, in_maps, core_ids=list(range(NCORES)))
    _BUILt["last_results"] = res

    b_o1 = np.asarray(inputs["b_o1"], np.float32)
    outs = []
    for br in range(3):
        ob = np.zeros((B, 256, H, W), np.float32)
        for b in range(B):
            acc = (np.asarray(res.results[2 * b]["o"][br], np.float32)
                   + np.asarray(res.results[2 * b + 1]["o"][br], np.float32))
            ob[b] = (acc.reshape(256, W, H).transpose(0, 2, 1)
                     + b_o1[:, None, None])
        outs.append(ob)
    return tuple(outs)


# revision 11
# speedup vs baseline: 1.1756x; 1.1756x over previous
"""AugmentedConv attention kernel for 8 TRN2 NeuronCores.

Sharding: core c = 2*b + g  (b = batch 0..3, g = head-group 0..1, 4 heads each).
Each core: 3x conv3x3 (its 384 qkv output channels, full 256-ch contraction),
per-head attention with relative-position logits folded into the logit matmul
as extra accumulating passes (one-hot lhsT), softmax without max-subtraction
(logits are O(+-8)), AV + row-sum matmuls, 1x1 out-conv partial over its 128
v-channels.  Host sums the two partials per batch and adds b_o1.

Layouts (per core):
  spatial index s = y*32 + x  (row-major, matches reference H*W flatten)
  q,k stored [c=128 (4 heads x 32), s=1024] channel-major (bf16)
  v transposed on-device to [s, c] for the AV contraction
  logits computed TRANSPOSED as l[k, q] so that softmax-sum and AV contract
  over the partition dim via matmuls (row-tiled, 4 heads packed).
"""

import sys

sys.path.insert(0, "/opt/trn_rl_repo")

import numpy as np
import ml_dtypes

BF16 = ml_dtypes.bfloat16

B, C, H, W = 4, 256, 32, 32
DK = DV = 256
NH = 8
DKH = DVH = 32
K = 3
HW = H * W  # 1024
NCORES = 8
HEADS_PER_CORE = 4
SCALE = DKH ** -0.5

_BUILt = {}


def _build_nc():
    import concourse.bass as bass
    import concourse.mybir as mybir
    import concourse.tile as tile
    from concourse import bacc
    from concourse.masks import make_identity

    f32 = mybir.dt.float32
    bf16 = mybir.dt.bfloat16
    AF = mybir.ActivationFunctionType
    ALU = mybir.AluOpType

    nc = bacc.Bacc("TRN2", target_bir_lowering=False, debug=False,
                   num_devices=NCORES)

    # ---- external I/O (per-core shards, host-prepped) ----
    xp_d = nc.declare_dram_parameter("xp", [3, 2, 128, 34 * 34], bf16, isOutput=False)
    wt_d = nc.declare_dram_parameter("wt", [3, 2, 128, 9, 384], bf16, isOutput=False)
    bqkv_d = nc.declare_dram_parameter("bqkv", [128, 3, 3], f32, isOutput=False)
    woT_d = nc.declare_dram_parameter("woT", [128, 256], bf16, isOutput=False)
    relh_d = nc.declare_dram_parameter("relh", [32, 128, 32], bf16, isOutput=False)
    relw_d = nc.declare_dram_parameter("relw", [32, 128, 32], bf16, isOutput=False)
    ohy_d = nc.declare_dram_parameter("ohy", [128, 1024], bf16, isOutput=False)
    ohx_d = nc.declare_dram_parameter("ohx", [128, 1024], bf16, isOutput=False)
    o_d = nc.declare_dram_parameter("o", [3, 256, 1024], f32, isOutput=True)

    with tile.TileContext(nc) as tc:
        with (
            tc.tile_pool(name="const", bufs=1) as const,
            tc.tile_pool(name="work", bufs=1) as work,
            tc.tile_pool(name="cpsum", bufs=2, space="PSUM") as cpsum,
            tc.tile_pool(name="lpsum", bufs=1, space="PSUM") as lpsum,
            tc.tile_pool(name="apsum", bufs=1, space="PSUM") as apsum,
            tc.tile_pool(name="expp", bufs=4) as expp,
            tc.tile_pool(name="outp", bufs=2) as outp,
            tc.tile_pool(name="dscr", bufs=1, space="DRAM") as dscr,
        ):
            # ---- constants ----
            bqkv = const.tile([128, 3, 3], f32)
            nc.sync.dma_start(bqkv[:], bqkv_d[:])
            woT = const.tile([128, 256], bf16)
            nc.sync.dma_start(woT[:], woT_d[:])
            relh = const.tile([128, 32, 32], bf16)
            nc.sync.dma_start(relh[:], relh_d[:].rearrange("y p m -> p y m"))
            relw = const.tile([128, 32, 32], bf16)
            nc.sync.dma_start(relw[:], relw_d[:].rearrange("y p m -> p y m"))
            ohy = const.tile([128, 1024], bf16)
            nc.sync.dma_start(ohy[:], ohy_d[:])
            ohx = const.tile([128, 1024], bf16)
            nc.sync.dma_start(ohx[:], ohx_d[:])
            ident = const.tile([128, 128], bf16)
            make_identity(nc, ident[:])
            ones32 = const.tile([128, 32], bf16)
            nc.vector.memset(ones32[:], 1.0)

            xp = const.tile([128, 3, 2, 34 * 34], bf16)
            wt = const.tile([128, 3, 2, 9, 384], bf16)
            xp_loaded = [False] * 3

            def load_t(t):
                if not xp_loaded[t]:
                    xp_loaded[t] = True
                    for cib in range(2):
                        nc.sync.dma_start(xp[:, t, cib], xp_d[t, cib])
                        nc.sync.dma_start(wt[:, t, cib], wt_d[t, cib])

            qkvt = [[work.tile([128, 1024], bf16, name=f"qkv_{t}_{ct}")
                     for ct in range(3)] for t in range(3)]
            vTt = [work.tile([128, 8, 4, 32], bf16, name=f"vT_{t}")
                   for t in range(3)]
            Rh = work.tile([128, 1024], bf16)
            Rw = work.tile([128, 1024], bf16)
            kd = work.tile([128, 1024], bf16)
            scr = dscr.tile([3, 4, 32, 32, 32], bf16)

            xpv = xp[:].rearrange("p t c (r q) -> p t c r q", q=34)

            def conv_job(t, cot, qt):
                load_t(t)
                ps = cpsum.tile([128, 512], f32, tag="conv")
                n = 0
                for cib in range(2):
                    for ky in range(3):
                        for kx in range(3):
                            rhs = xpv[:, t, cib,
                                      qt * 16 + ky: qt * 16 + ky + 16,
                                      kx: kx + 32]
                            nc.tensor.matmul(
                                ps[:],
                                lhsT=wt[:, t, cib, ky * 3 + kx,
                                        cot * 128:(cot + 1) * 128],
                                rhs=rhs,
                                start=(n == 0), stop=(n == 17),
                            )
                            n += 1
                nc.vector.tensor_tensor(
                    out=qkvt[t][cot][:, qt * 512:(qt + 1) * 512],
                    in0=ps[:],
                    in1=bqkv[:, t, cot, None].to_broadcast((128, 512)),
                    op=ALU.add,
                )

            def vt_job(t, half):
                for kt in range(4 * half, 4 * half + 4):
                    pst = cpsum.tile([128, 128], bf16, tag="conv")
                    nc.tensor.transpose(
                        pst[:],
                        qkvt[t][2][:, kt * 128:(kt + 1) * 128],
                        ident[:],
                    )
                    nc.vector.tensor_copy(
                        out=vTt[t][:, kt, :, :].rearrange("p a b -> p (a b)"),
                        in_=pst[:],
                    )

            def rel_job(kind, j):
                """one head's rel table: 2x16-col-group accumulation sweeps"""
                q1 = qkvt[0][0][:]
                q1v = qkvt[0][0][:].rearrange("p (y x) -> p y x", x=32)
                Rwv = Rw[:].rearrange("p (y x) -> p x y", x=32)
                for half in range(2):
                    ps = cpsum.tile([128, 512], f32, tag="conv")
                    for yl in range(16):
                        y = half * 16 + yl
                        if kind == 0:
                            lhs = relh[32 * j:32 * j + 32, y, :]
                            rhs = q1[32 * j:32 * j + 32, y * 32:y * 32 + 32]
                        else:
                            lhs = relw[32 * j:32 * j + 32, y, :]
                            rhs = q1v[32 * j:32 * j + 32, :, y]
                        nc.tensor.matmul(
                            ps[32 * j:32 * j + 32, yl * 32:yl * 32 + 32],
                            lhsT=lhs, rhs=rhs,
                            tile_position=(32 * j, 32 * j),
                            start=(yl == 0), stop=(yl == 15),
                        )
                    if kind == 0:
                        nc.vector.tensor_copy(
                            out=Rh[32 * j:32 * j + 32,
                                   half * 512:half * 512 + 512],
                            in_=ps[32 * j:32 * j + 32, :],
                        )
                    else:
                        nc.vector.tensor_copy(
                            out=Rwv[32 * j:32 * j + 32,
                                    half * 16:half * 16 + 16, :],
                            in_=ps[32 * j:32 * j + 32, :].rearrange(
                                "p (a b) -> p a b", b=32),
                        )

            def kd_job():
                nc.vector.tensor_tensor(out=kd[:], in0=qkvt[0][1][:],
                                        in1=qkvt[1][1][:], op=ALU.subtract)

            def attention(br, qs, ks, vt, userel, fillers):
                fi = 0
                for qt in range(2):
                    pa = apsum.tile([128, 512], f32, tag="av")
                    psum_s = apsum.tile([128, 512], f32, tag="sums")
                    lagged = []

                    def emit_av(exs, kt):
                        for hp in range(2):
                            for jl, j in enumerate((2 * hp, 2 * hp + 1)):
                                nc.tensor.matmul(
                                    pa[32 * j:32 * j + 32, :],
                                    lhsT=vTt[vt][:, kt, j, :],
                                    rhs=exs[hp][:, jl, :],
                                    tile_position=(0, 32 * j),
                                    start=(kt == 0), stop=(kt == 7),
                                )
                                nc.tensor.matmul(
                                    psum_s[32 * j:32 * j + 32, :],
                                    lhsT=ones32[:],
                                    rhs=exs[hp][:, jl, :],
                                    tile_position=(0, 32 * j),
                                    start=(kt == 0), stop=(kt == 7),
                                )
                    for kt in range(8):
                        pl = lpsum.tile([128, 4, 512], f32, tag="logits")
                        for j in range(4):
                            nc.tensor.matmul(
                                pl[:, j, :],
                                lhsT=ks[32 * j:32 * j + 32,
                                        kt * 128:(kt + 1) * 128],
                                rhs=qs[32 * j:32 * j + 32,
                                       qt * 512:(qt + 1) * 512],
                                tile_position=(32 * j, 0),
                                start=True, stop=not userel,
                            )
                        if userel:
                            for j in range(4):
                                nc.tensor.matmul(
                                    pl[:, j, :],
                                    lhsT=ohy[32 * j:32 * j + 32,
                                             kt * 128:(kt + 1) * 128],
                                    rhs=Rh[32 * j:32 * j + 32,
                                           qt * 512:(qt + 1) * 512],
                                    tile_position=(32 * j, 0),
                                    start=False, stop=False,
                                )
                            for j in range(4):
                                nc.tensor.matmul(
                                    pl[:, j, :],
                                    lhsT=ohx[32 * j:32 * j + 32,
                                             kt * 128:(kt + 1) * 128],
                                    rhs=Rw[32 * j:32 * j + 32,
                                           qt * 512:(qt + 1) * 512],
                                    tile_position=(32 * j, 0),
                                    start=False, stop=True,
                                )
                        exs = []
                        for hp in range(2):
                            ex = expp.tile([128, 2, 512], bf16, tag="exp")
                            nc.scalar.activation(
                                ex[:], pl[:, 2 * hp:2 * hp + 2, :], AF.Exp)
                            exs.append(ex)
                        # av/sums lag one kt so next logits aren't stuck
                        # behind them on the PE stream
                        lagged.append((exs, kt))
                        if len(lagged) > 1:
                            emit_av(*lagged.pop(0))
                        # filler work for PE while ScalarE runs exp
                        if fi < len(fillers):
                            fillers[fi]()
                            fi += 1
                    emit_av(*lagged.pop(0))
                    recip = outp.tile([128, 512], f32, tag="recip")
                    nc.vector.reciprocal_approx_fast(out=recip[:],
                                                     in_=psum_s[:])
                    a_sb = outp.tile([128, 512], bf16, tag="asb")
                    nc.vector.tensor_tensor(out=a_sb[:], in0=pa[:],
                                            in1=recip[:], op=ALU.mult)
                    nc.sync.dma_start(
                        out=scr[br].rearrange("n x dp y -> (n x) dp y")[
                            :, qt * 16:qt * 16 + 16, :],
                        in_=a_sb[:].rearrange("p (dp y) -> p dp y", y=32),
                    )
                while fi < len(fillers):
                    fillers[fi]()
                    fi += 1
                G = outp.tile([128, 1024], bf16, tag="G")
                for n in range(4):
                    nc.sync.dma_start(
                        out=G[32 * n:32 * n + 32, :].rearrange(
                            "p (x y) -> p x y", y=32),
                        in_=scr[br, n].rearrange("x dp y -> dp x y"),
                    )
                for qt2 in range(2):
                    for cot in range(2):
                        po = cpsum.tile([128, 512], f32, tag="conv")
                        nc.tensor.matmul(
                            po[:], lhsT=woT[:, cot * 128:(cot + 1) * 128],
                            rhs=G[:, qt2 * 512:(qt2 + 1) * 512],
                            start=True, stop=True)
                        osb = outp.tile([128, 512], f32, tag="osb")
                        nc.vector.tensor_copy(out=osb[:], in_=po[:])
                        nc.sync.dma_start(
                            out=o_d[br, cot * 128:(cot + 1) * 128,
                                    qt2 * 512:(qt2 + 1) * 512],
                            in_=osb[:],
                        )

            # ---- schedule: prologue convs, then branches 1, 2, 0 with
            # remaining conv/rel work as exp-gap fillers ----
            for t, cot in [(1, 0), (2, 1), (1, 2)]:
                for qt in range(2):
                    conv_job(t, cot, qt)
            vt_job(1, 0)
            vt_job(1, 1)

            br1_fill = [
                lambda: conv_job(0, 0, 0), lambda: conv_job(0, 0, 1),  # q1
                lambda: rel_job(0, 0), lambda: rel_job(0, 1),
                lambda: rel_job(0, 2), lambda: rel_job(0, 3),
                lambda: rel_job(1, 0), lambda: rel_job(1, 1),
                lambda: rel_job(1, 2), lambda: rel_job(1, 3),
                lambda: conv_job(0, 2, 0), lambda: conv_job(0, 2, 1),  # v1
                lambda: vt_job(0, 0), lambda: vt_job(0, 1),
            ]
            attention(1, qkvt[1][0][:], qkvt[2][1][:], 1, False, br1_fill)

            br0_fill = [
                lambda: conv_job(0, 1, 0), lambda: conv_job(0, 1, 1),  # k1
                lambda: conv_job(1, 1, 0), lambda: conv_job(1, 1, 1),  # k2
                kd_job,
                lambda: conv_job(2, 0, 0), lambda: conv_job(2, 0, 1),  # q12
                lambda: conv_job(2, 2, 0), lambda: conv_job(2, 2, 1),  # v12
                lambda: vt_job(2, 0), lambda: vt_job(2, 1),
            ]
            attention(0, qkvt[0][0][:], qkvt[2][1][:], 0, True, br0_fill)

            attention(2, qkvt[2][0][:], kd[:], 2, False, [])

    nc.compile()
    return nc


def _host_prep(inputs):
    """Build per-core in_maps."""
    x_all = [inputs["x1"], inputs["x2"], inputs["x12"]]
    w_all = [inputs["W_qkv1"], inputs["W_qkv2"], inputs["W_qkv12"]]
    b_all = [inputs["b_qkv1"], inputs["b_qkv2"], inputs["b_qkv12"]]
    W_o1 = np.asarray(inputs["W_o1"], np.float32)
    key_rel_w = np.asarray(inputs["key_rel_w"], np.float32)
    key_rel_h = np.asarray(inputs["key_rel_h"], np.float32)

    # padded inputs, bf16: [B, 2, 128, 34*34]
    xp_full = []
    for xt in x_all:
        xt = np.asarray(xt, np.float32)
        xpad = np.zeros((B, C, 34, 34), np.float32)
        xpad[:, :, 1:33, 1:33] = xt
        xp_full.append(xpad.reshape(B, 2, 128, 34 * 34))
    xp_full = np.stack(xp_full, axis=1)  # [B, 3, 2, 128, 1156]

    # one-hots / e4, shared by all cores
    kk = np.arange(1024)
    ohy = np.zeros((128, 1024), np.float32)
    ohx = np.zeros((128, 1024), np.float32)
    for j in range(4):
        for i in range(32):
            ohy[32 * j + i] = (kk // 32 == i)
            ohx[32 * j + i] = (kk % 32 == i)
    # rel tables (shared): relh[y, 32j+d, i] = key_rel_h[i - y + 31, d]
    relh = np.zeros((32, 128, 32), np.float32)
    relw = np.zeros((32, 128, 32), np.float32)
    for y in range(32):
        blk_h = key_rel_h[31 - y:63 - y, :].T  # [d=32, i=32]
        blk_w = key_rel_w[31 - y:63 - y, :].T
        for j in range(4):
            relh[y, 32 * j:32 * j + 32, :] = blk_h
            relw[y, 32 * j:32 * j + 32, :] = blk_w

    in_maps = []
    for c in range(NCORES):
        b, g = divmod(c, 2)
        # conv weights: 384 rows = (q128 scaled, k128, v128) for head group g
        wts = []
        bias = np.zeros((128, 3, 3), np.float32)
        for t in range(3):
            wf = np.asarray(w_all[t], np.float32)
            bf = np.asarray(b_all[t], np.float32)
            wq = wf[128 * g:128 * g + 128] * SCALE
            wk = wf[256 + 128 * g:256 + 128 * g + 128]
            wv = wf[512 + 128 * g:512 + 128 * g + 128]
            wsl = np.concatenate([wq, wk, wv], 0)  # [384, 256, 3, 3]
            # -> [2 cib, 128 ci, 9 tap, 384 co]
            wts.append(wsl.reshape(384, 2, 128, 9).transpose(1, 2, 3, 0))
            bias[:, t, 0] = bf[128 * g:128 * g + 128] * SCALE
            bias[:, t, 1] = bf[256 + 128 * g:256 + 128 * g + 128]
            bias[:, t, 2] = bf[512 + 128 * g:512 + 128 * g + 128]
        wt = np.stack(wts, 0)  # [3, 2, 128, 9, 384]
        woT = W_o1[:, 128 * g:128 * g + 128, 0, 0].T  # [128 dv, 256 co]
        in_maps.append({
            "xp": xp_full[b].astype(BF16),
            "wt": wt.astype(BF16),
            "bqkv": bias,
            "woT": np.ascontiguousarray(woT).astype(BF16),
            "relh": relh.astype(BF16),
            "relw": relw.astype(BF16),
            "ohy": ohy.astype(BF16),
            "ohx": ohx.astype(BF16),
        })
    return in_maps


def kernel(**inputs):
    from concourse.bass_utils import run_bass_kernel_spmd

    if "nc" not in _BUILt:
        _BUILt["nc"] = _build_nc()
    nc = _BUILt["nc"]

    in_maps = _host_prep(inputs)
    res = run_bass_kernel_spmd(nc, in_maps, core_ids=list(range(NCORES)))
    _BUILt["last_results"] = res

    b_o1 = np.asarray(inputs["b_o1"], np.float32)
    outs = []
    for br in range(3):
        ob = np.zeros((B, 256, H, W), np.float32)
        for b in range(B):
            acc = (np.asarray(res.results[2 * b]["o"][br], np.float32)
                   + np.asarray(res.results[2 * b + 1]["o"][br], np.float32))
            ob[b] = (acc.reshape(256, W, H).transpose(0, 2, 1)
                     + b_o1[:, None, None])
        outs.append(ob)
    return tuple(outs)
